# revision 9
# baseline (speedup 1.0000x reference)
"""Gromov-Wasserstein embedding loss kernel for 8x TRN2 NeuronCores.

Math (see reference):
  cos[i,j]  = (e1[i] . e2[j]) / (|e1[i]| |e2[j]| + eps)
  cost      = 1 - exp(cos - 1)
  d_w       = sum(cost * trans) = sum(trans) - sum(exp(cos-1) * trans)
  reg       = |E1^T E1 - I|_F^2 + |E2^T E2 - I|_F^2
  out       = [d_w, reg]

Sharding: rows of trans / cos split 8 ways (1024 rows per core).

Key trick: trans is folded into the exp via logs. Host ships
lnU = ln(trans * 2^26) in fp8; on device a scaled identity matmul
(I*256, bf16) preloads 256*lnU into PSUM, fp8 DoubleRow matmuls
accumulate 256*cos on top (host pre-normalizes embeddings, scales by
16, and pre-transposes into DoubleRow [128,2,N] layout), and a single
ACT pass computes exp(psum/256 - 1) with accum_out, yielding
sum_j trans*exp(cos-1) per row-block with no vector-engine work at
all. PE also accumulates the 256x256 grams of the raw bf16 row shards
for the regularizer. Host sums the tiny partials.
"""

import sys

sys.path.insert(0, "/opt/trn_rl_repo")

import numpy as np

from concourse import bass, bacc, mybir
from concourse import tile
from concourse.bass_utils import run_bass_kernel_spmd

NCORES = 8
NUM = 8192
DIM = 256
SHARD = NUM // NCORES  # 1024 rows per core

BF16 = mybir.dt.bfloat16
F8 = mybir.dt.float8e4
F32 = mybir.dt.float32
NP_BF16 = mybir.dt.np(BF16)
NP_F8 = mybir.dt.np(F8)

LSCALE = 2.0**26  # trans prescale so ln(U) fits fp8 comfortably

_cached = {}


def build_program():
    nc = bacc.Bacc(None, target_bir_lowering=False)

    i2 = nc.declare_dram_parameter("i2", [128, 128], BF16, isOutput=False)
    cst = nc.declare_dram_parameter("cst", [128, 1], F32, isOutput=False)
    n1t = nc.declare_dram_parameter("n1t", [128, 2, SHARD], F8, isOutput=False)
    n2t = nc.declare_dram_parameter("n2t", [128, 2, NUM], F8, isOutput=False)
    lu = nc.declare_dram_parameter("lu", [SHARD, NUM], F8, isOutput=False)
    e1s = nc.declare_dram_parameter("e1s", [SHARD, DIM], BF16, isOutput=False)
    e2s = nc.declare_dram_parameter("e2s", [SHARD, DIM], BF16, isOutput=False)
    g1o = nc.declare_dram_parameter("g1", [DIM, DIM], F32, isOutput=True)
    g2o = nc.declare_dram_parameter("g2", [DIM, DIM], F32, isOutput=True)
    acco = nc.declare_dram_parameter("acc", [128, 32], F32, isOutput=True)

    AF = mybir.ActivationFunctionType
    DR = mybir.MatmulPerfMode.DoubleRow

    with tile.TileContext(nc) as tc:
        with (
            tc.tile_pool(name="const", bufs=1) as constp,
            tc.tile_pool(name="stats", bufs=1) as statsp,
        ):
            i2t = constp.tile([128, 128], BF16)
            nc.sync.dma_start(out=i2t[:], in_=i2[:, :])
            cstt = constp.tile([128, 1], F32)
            nc.sync.dma_start(out=cstt[:], in_=cst[:, :])
            neg1 = cstt[:, 0:1]
            # table stream on the Pool engine's queue, parallel to the lu
            # stream on SP's queue
            n1tt = constp.tile([128, 2, SHARD], F8)
            nc.sync.dma_start(out=n1tt[:], in_=n1t[:, :, :])
            n2tt = constp.tile([128, 2, NUM], F8)
            nc.sync.dma_start(
                out=n2tt[:, :, 0:2048],
                in_=n2t[:, :, 0:2048],
            )

            accs = statsp.tile([128, 32], F32)  # d_w partials
            warm = statsp.tile([128, 1], F32)
            # dummy activation pulls the Exp table load off the critical path
            nc.scalar.activation(warm[:, 0:1], cstt[:, 0:1], AF.Exp, bias=neg1)

            # ---------------- main loop: cos + exp + weighted reduce -------
            with (
                tc.tile_pool(name="lut", bufs=3) as lup,
                tc.tile_pool(name="grp", bufs=2) as grpp,
                tc.tile_pool(name="gdrain", bufs=1) as gdp,
                tc.tile_pool(name="psumB", bufs=2, space="PSUM") as pbp,
            ):
                for i in range(8):
                    for jg in range(4):
                        lut = lup.tile([128, 2048], F8, tag="lu")
                        nc.sync.dma_start(
                            out=lut[:],
                            in_=lu[i * 128 : (i + 1) * 128, jg * 2048 : (jg + 1) * 2048],
                        )
                        if i == 0 and jg < 3:
                            # stream the remaining cols of the emb2 table in
                            # on the table queue
                            q = jg + 1
                            nc.sync.dma_start(
                                out=n2tt[:, :, q * 2048 : (q + 1) * 2048],
                                in_=n2t[:, :, q * 2048 : (q + 1) * 2048],
                            )
                        ps = pbp.tile([128, 2048], F32, tag="ps")
                        # preload 256*lnU into each 512-col psum bank
                        for jj in range(4):
                            nc.tensor.matmul(
                                ps[:, jj * 512 : (jj + 1) * 512],
                                lhsT=i2t[:],
                                rhs=lut[:, jj * 512 : (jj + 1) * 512],
                                start=True,
                                stop=False,
                                skip_group_check=True,
                            )
                        # accumulate 256*cos (fp8 DoubleRow, K=256 per instr)
                        for jj in range(4):
                            n0 = jg * 2048 + jj * 512
                            nc.tensor.matmul(
                                ps[:, jj * 512 : (jj + 1) * 512],
                                lhsT=n1tt[:, :, i * 128 : (i + 1) * 128],
                                rhs=n2tt[:, :, n0 : n0 + 512],
                                start=False,
                                stop=True,
                                perf_mode=DR,
                                skip_group_check=True,
                            )
                        # exp(psum/256 - 1) = trans*2^26 * exp(cos-1), written
                        # back in place; accum_out row-reduces it for free
                        nc.scalar.activation(
                            ps[:],
                            ps[:],
                            AF.Exp,
                            bias=neg1,
                            scale=1.0 / 256.0,
                            accum_out=accs[:, i * 4 + jg : i * 4 + jg + 1],
                        )

                # ---- grams of raw shards (regularizer), in the ACT tail ----
                # quarters live in the 4 banks of one more rotating psum tile
                psg = pbp.tile([128, 2048], F32, tag="ps")
                for gi, src in ((0, e1s), (2, e2s)):
                    grp = grpp.tile([128, 8, DIM], BF16, tag="grp")
                    for k in range(8):
                        nc.sync.dma_start(
                            out=grp[:, k, :], in_=src[k * 128 : (k + 1) * 128, :]
                        )
                    for k in range(8):
                        first = k == 0
                        last = k == 7
                        nc.tensor.matmul(
                            psg[:, gi * 512 : gi * 512 + DIM],
                            lhsT=grp[:, k, 0:128],
                            rhs=grp[:, k, :],
                            start=first,
                            stop=last,
                            skip_group_check=True,
                        )
                        nc.tensor.matmul(
                            psg[:, (gi + 1) * 512 : (gi + 1) * 512 + DIM],
                            lhsT=grp[:, k, 128:256],
                            rhs=grp[:, k, :],
                            start=first,
                            stop=last,
                            skip_group_check=True,
                        )

                gsb = gdp.tile([128, 4 * DIM], F32)
                dsts = (g1o[0:128, :], g1o[128:256, :], g2o[0:128, :], g2o[128:256, :])
                for q in range(4):
                    nc.vector.tensor_copy(
                        gsb[:, q * DIM : (q + 1) * DIM],
                        psg[:, q * 512 : q * 512 + DIM],
                    )
                    nc.sync.dma_start(
                        out=dsts[q], in_=gsb[:, q * DIM : (q + 1) * DIM]
                    )

            nc.sync.dma_start(out=acco[:, :], in_=accs[:])

    nc.finalize()
    return nc


def prepare(inputs):
    """Build (cached) program + per-core input maps. Returns (nc, in_maps, st)."""
    index1 = inputs["index1"]
    index2 = inputs["index2"]
    trans = inputs["trans"]
    emb1_w = inputs["emb1_w"]
    emb2_w = inputs["emb2_w"]
    # gather (identity for arange inputs, but stay correct in general)
    e1 = np.asarray(emb1_w, dtype=np.float32)[np.asarray(index1).astype(np.int64)]
    e2 = np.asarray(emb2_w, dtype=np.float32)[np.asarray(index2).astype(np.int64)]
    trans = np.ascontiguousarray(np.asarray(trans, dtype=np.float32))

    # sum(trans) on host (float64 accumulate)
    st = float(trans.sum(dtype=np.float64))

    # normalized, x16-scaled, fp8, transposed into DoubleRow [128, 2, N] layout
    def prep_table(e):
        n = e / (np.sqrt((e.astype(np.float64) ** 2).sum(1, keepdims=True)) + 1e-16)
        q = (n.astype(np.float32) * 16.0).astype(NP_F8)  # [N, 256]
        return np.ascontiguousarray(q.T.reshape(2, 128, -1).transpose(1, 0, 2))

    n1T = prep_table(e1)  # [128, 2, NUM]
    n2T = prep_table(e2)

    # ln(trans * 2^26) in fp8 (clipped; exp() recovers trans*2^26)
    U = trans * np.float32(LSCALE)
    lnU = np.log(np.maximum(U, np.float32(1e-30)))
    np.maximum(lnU, np.float32(-50.0), out=lnU)
    lnU8 = lnU.astype(NP_F8)

    e1b = np.ascontiguousarray(e1.astype(NP_BF16))
    e2b = np.ascontiguousarray(e2.astype(NP_BF16))

    if "nc" not in _cached:
        _cached["nc"] = build_program()
    nc = _cached["nc"]

    i2 = (np.eye(128, dtype=np.float32) * 256.0).astype(NP_BF16)
    cstv = np.full((128, 1), -1.0, dtype=np.float32)
    in_maps = []
    for c in range(NCORES):
        in_maps.append(
            {
                "i2": i2,
                "cst": cstv,
                "n1t": np.ascontiguousarray(n1T[:, :, c * SHARD : (c + 1) * SHARD]),
                "n2t": n2T,
                "lu": lnU8[c * SHARD : (c + 1) * SHARD],
                "e1s": e1b[c * SHARD : (c + 1) * SHARD],
                "e2s": e2b[c * SHARD : (c + 1) * SHARD],
            }
        )
    return nc, in_maps, st


def kernel(index1, index2, trans, emb1_w, emb2_w):
    nc, in_maps, st = prepare(
        dict(index1=index1, index2=index2, trans=trans, emb1_w=emb1_w, emb2_w=emb2_w)
    )

    res = run_bass_kernel_spmd(nc, in_maps, list(range(NCORES)))
    results = res.results

    syt = 0.0
    G1 = np.zeros((DIM, DIM), dtype=np.float64)
    G2 = np.zeros((DIM, DIM), dtype=np.float64)
    for c in range(NCORES):
        syt += float(results[c]["acc"].sum(dtype=np.float64))
        G1 += results[c]["g1"].astype(np.float64)
        G2 += results[c]["g2"].astype(np.float64)

    d_w = st - syt / LSCALE
    eye = np.eye(DIM, dtype=np.float64)
    reg = ((G1 - eye) ** 2).sum() + ((G2 - eye) ** 2).sum()
    return np.array([d_w, reg], dtype=np.float32)


# revision 14
# speedup vs baseline: 1.0281x; 1.0281x over previous
"""Gromov-Wasserstein embedding loss kernel for 8x TRN2 NeuronCores.

Math (see reference):
  cos[i,j]  = (e1[i] . e2[j]) / (|e1[i]| |e2[j]| + eps)
  cost      = 1 - exp(cos - 1)
  d_w       = sum(cost * trans) = sum(trans) - sum(exp(cos-1) * trans)
  reg       = |E1^T E1 - I|_F^2 + |E2^T E2 - I|_F^2
  out       = [d_w, reg]

Sharding: rows of trans / cos split 8 ways (1024 rows per core).

Key trick: trans is folded into the exp via logs. Host ships
lnU = ln(trans * 2^26) in fp8; on device a scaled identity matmul
(I*256, bf16) preloads 256*lnU into PSUM, fp8 DoubleRow matmuls
accumulate 256*cos on top (host pre-normalizes embeddings, scales by
16, and pre-transposes into DoubleRow [128,2,N] layout), and a single
ACT pass computes exp(psum/256 - 1) with accum_out, yielding
sum_j trans*exp(cos-1) per row-block with no vector-engine work at
all. PE also accumulates the 256x256 grams of the raw bf16 row shards
for the regularizer. Host sums the tiny partials.
"""

import sys

sys.path.insert(0, "/opt/trn_rl_repo")

import numpy as np

from concourse import bass, bacc, mybir
from concourse import tile
from concourse.bass_utils import run_bass_kernel_spmd

NCORES = 8
NUM = 8192
DIM = 256
SHARD = NUM // NCORES  # 1024 rows per core

BF16 = mybir.dt.bfloat16
F8 = mybir.dt.float8e4
F32 = mybir.dt.float32
NP_BF16 = mybir.dt.np(BF16)
NP_F8 = mybir.dt.np(F8)

LSCALE = 2.0**26  # trans prescale so ln(U) fits fp8 comfortably

_cached = {}


def build_program():
    nc = bacc.Bacc(None, target_bir_lowering=False)

    i2 = nc.declare_dram_parameter("i2", [128, 128], BF16, isOutput=False)
    cst = nc.declare_dram_parameter("cst", [128, 1], F32, isOutput=False)
    n1t = nc.declare_dram_parameter("n1t", [128, 2, SHARD], F8, isOutput=False)
    n2t = nc.declare_dram_parameter("n2t", [128, 2, NUM], F8, isOutput=False)
    lu = nc.declare_dram_parameter("lu", [SHARD, NUM], F8, isOutput=False)
    e1s = nc.declare_dram_parameter("e1s", [128, 4, 2, DIM], F8, isOutput=False)
    e2s = nc.declare_dram_parameter("e2s", [128, 4, 2, DIM], F8, isOutput=False)
    g1o = nc.declare_dram_parameter("g1", [DIM, DIM], F32, isOutput=True)
    g2o = nc.declare_dram_parameter("g2", [DIM, DIM], F32, isOutput=True)
    acco = nc.declare_dram_parameter("acc", [128, 32], F32, isOutput=True)

    AF = mybir.ActivationFunctionType
    DR = mybir.MatmulPerfMode.DoubleRow

    with tile.TileContext(nc) as tc:
        with (
            tc.tile_pool(name="const", bufs=1) as constp,
            tc.tile_pool(name="stats", bufs=1) as statsp,
        ):
            i2t = constp.tile([128, 128], BF16)
            nc.sync.dma_start(out=i2t[:], in_=i2[:, :])
            cstt = constp.tile([128, 1], F32)
            nc.sync.dma_start(out=cstt[:], in_=cst[:, :])
            neg1 = cstt[:, 0:1]
            # table stream on the Pool engine's queue, parallel to the lu
            # stream on SP's queue
            n1tt = constp.tile([128, 2, SHARD], F8)
            nc.sync.dma_start(out=n1tt[:], in_=n1t[:, :, :])
            n2tt = constp.tile([128, 2, NUM], F8)
            nc.sync.dma_start(
                out=n2tt[:, :, 0:2048],
                in_=n2t[:, :, 0:2048],
            )

            accs = statsp.tile([128, 32], F32)  # d_w partials
            warm = statsp.tile([128, 1], F32)
            # dummy activation pulls the Exp table load off the critical path
            nc.scalar.activation(warm[:, 0:1], cstt[:, 0:1], AF.Exp, bias=neg1)

            # ---------------- main loop: cos + exp + weighted reduce -------
            with (
                tc.tile_pool(name="lut", bufs=3) as lup,
                tc.tile_pool(name="grp", bufs=2) as grpp,
                tc.tile_pool(name="gdrain", bufs=1) as gdp,
                tc.tile_pool(name="psumB", bufs=2, space="PSUM") as pbp,
            ):
                for jg in range(4):
                    for i in range(8):
                        lut = lup.tile([128, 2048], F8, tag="lu")
                        nc.sync.dma_start(
                            out=lut[:],
                            in_=lu[i * 128 : (i + 1) * 128, jg * 2048 : (jg + 1) * 2048],
                        )
                        if i == 0 and jg < 3:
                            # next emb2-table chunk streams in behind this
                            # column group's first lnU tile; 8 tiles of work
                            # hide its latency
                            q = jg + 1
                            nc.sync.dma_start(
                                out=n2tt[:, :, q * 2048 : (q + 1) * 2048],
                                in_=n2t[:, :, q * 2048 : (q + 1) * 2048],
                            )
                        ps = pbp.tile([128, 2048], F32, tag="ps")
                        # preload 256*lnU into each 512-col psum bank
                        for jj in range(4):
                            nc.tensor.matmul(
                                ps[:, jj * 512 : (jj + 1) * 512],
                                lhsT=i2t[:],
                                rhs=lut[:, jj * 512 : (jj + 1) * 512],
                                start=True,
                                stop=False,
                                skip_group_check=True,
                            )
                        # accumulate 256*cos (fp8 DoubleRow, K=256 per instr)
                        for jj in range(4):
                            n0 = jg * 2048 + jj * 512
                            nc.tensor.matmul(
                                ps[:, jj * 512 : (jj + 1) * 512],
                                lhsT=n1tt[:, :, i * 128 : (i + 1) * 128],
                                rhs=n2tt[:, :, n0 : n0 + 512],
                                start=False,
                                stop=True,
                                perf_mode=DR,
                                skip_group_check=True,
                            )
                        # exp(psum/256 - 1) = trans*2^26 * exp(cos-1), written
                        # back in place; accum_out row-reduces it for free
                        nc.scalar.activation(
                            ps[:],
                            ps[:],
                            AF.Exp,
                            bias=neg1,
                            scale=1.0 / 256.0,
                            accum_out=accs[:, jg * 8 + i : jg * 8 + i + 1],
                        )

                # ---- grams of raw shards (regularizer), in the ACT tail ----
                # fp8 DoubleRow (x256-scaled shards from host); quarters live
                # in the 4 banks of one more rotating psum tile
                psg = pbp.tile([128, 2048], F32, tag="ps")
                gsb = gdp.tile([128, 4 * DIM], F32)
                dsts = (g1o[0:128, :], g1o[128:256, :], g2o[0:128, :], g2o[128:256, :])
                for gi, src in ((0, e1s), (2, e2s)):
                    grp = grpp.tile([128, 4, 2, DIM], F8, tag="grp")
                    nc.sync.dma_start(out=grp[:], in_=src[:, :, :, :])
                    for h in range(2):
                        q = gi + h
                        for b in range(4):
                            nc.tensor.matmul(
                                psg[:, q * 512 : q * 512 + DIM],
                                lhsT=grp[:, b, :, h * 128 : (h + 1) * 128],
                                rhs=grp[:, b, :, :],
                                start=(b == 0),
                                stop=(b == 3),
                                perf_mode=DR,
                                skip_group_check=True,
                            )
                        nc.vector.tensor_copy(
                            gsb[:, q * DIM : (q + 1) * DIM],
                            psg[:, q * 512 : q * 512 + DIM],
                        )
                        nc.sync.dma_start(
                            out=dsts[q], in_=gsb[:, q * DIM : (q + 1) * DIM]
                        )

            nc.sync.dma_start(out=acco[:, :], in_=accs[:])

    nc.finalize()
    return nc


def prepare(inputs):
    """Build (cached) program + per-core input maps. Returns (nc, in_maps, st)."""
    index1 = inputs["index1"]
    index2 = inputs["index2"]
    trans = inputs["trans"]
    emb1_w = inputs["emb1_w"]
    emb2_w = inputs["emb2_w"]
    # gather (identity for arange inputs, but stay correct in general)
    e1 = np.asarray(emb1_w, dtype=np.float32)[np.asarray(index1).astype(np.int64)]
    e2 = np.asarray(emb2_w, dtype=np.float32)[np.asarray(index2).astype(np.int64)]
    trans = np.ascontiguousarray(np.asarray(trans, dtype=np.float32))

    # sum(trans) on host (float64 accumulate)
    st = float(trans.sum(dtype=np.float64))

    # normalized, x16-scaled, fp8, transposed into DoubleRow [128, 2, N] layout
    def prep_table(e):
        n = e / (np.sqrt((e.astype(np.float64) ** 2).sum(1, keepdims=True)) + 1e-16)
        q = (n.astype(np.float32) * 16.0).astype(NP_F8)  # [N, 256]
        return np.ascontiguousarray(q.T.reshape(2, 128, -1).transpose(1, 0, 2))

    n1T = prep_table(e1)  # [128, 2, NUM]
    n2T = prep_table(e2)

    # ln(trans * 2^26) in fp8 (clipped; exp() recovers trans*2^26)
    U = trans * np.float32(LSCALE)
    lnU = np.log(np.maximum(U, np.float32(1e-30)))
    np.maximum(lnU, np.float32(-50.0), out=lnU)
    lnU8 = lnU.astype(NP_F8)

    # x256-scaled fp8 shards in DoubleRow layout [128, 4, 2, 256] per core
    # (gram comes back x65536; host rescales)
    def prep_gram(e, c):
        q = (e[c * SHARD : (c + 1) * SHARD] * 256.0).astype(NP_F8)
        return np.ascontiguousarray(q.reshape(4, 2, 128, DIM).transpose(2, 0, 1, 3))

    e1b = [prep_gram(e1, c) for c in range(NCORES)]
    e2b = [prep_gram(e2, c) for c in range(NCORES)]

    if "nc" not in _cached:
        _cached["nc"] = build_program()
    nc = _cached["nc"]

    i2 = (np.eye(128, dtype=np.float32) * 256.0).astype(NP_BF16)
    cstv = np.full((128, 1), -1.0, dtype=np.float32)
    in_maps = []
    for c in range(NCORES):
        in_maps.append(
            {
                "i2": i2,
                "cst": cstv,
                "n1t": np.ascontiguousarray(n1T[:, :, c * SHARD : (c + 1) * SHARD]),
                "n2t": n2T,
                "lu": lnU8[c * SHARD : (c + 1) * SHARD],
                "e1s": e1b[c],
                "e2s": e2b[c],
            }
        )
    return nc, in_maps, st


def kernel(index1, index2, trans, emb1_w, emb2_w):
    nc, in_maps, st = prepare(
        dict(index1=index1, index2=index2, trans=trans, emb1_w=emb1_w, emb2_w=emb2_w)
    )

    res = run_bass_kernel_spmd(nc, in_maps, list(range(NCORES)))
    results = res.results

    syt = 0.0
    G1 = np.zeros((DIM, DIM), dtype=np.float64)
    G2 = np.zeros((DIM, DIM), dtype=np.float64)
    for c in range(NCORES):
        syt += float(results[c]["acc"].sum(dtype=np.float64))
        G1 += results[c]["g1"].astype(np.float64)
        G2 += results[c]["g2"].astype(np.float64)
    G1 /= 65536.0
    G2 /= 65536.0

    d_w = st - syt / LSCALE
    eye = np.eye(DIM, dtype=np.float64)
    reg = ((G1 - eye) ** 2).sum() + ((G2 - eye) ** 2).sum()
    return np.array([d_w, reg], dtype=np.float32)
